# revision 1
# baseline (speedup 1.0000x reference)
"""Trainium2 Bass kernel for nn_AttentionNet (audio-visual attention).

Data-parallel across 8 NeuronCores: batch B=256 split 32 per core, i.e.
320 (b,t) rows and 320*49 = 15680 visual rows per core.

Per-core math (n indexes the 320 rows, s in [0,49), d/e in [0,512)):
    a_t = relu(audio @ Wa.T + ba)            [N,512]
    v_t = relu(vis @ Wv.T + bv)              [N,49,512]
    a_s = a_t @ Aa.T                         [N,49]
    v_s = v_t @ Av.T                         [N,49,49]
    f   = (tanh(a_s[:,:,None] + v_s)) @ Af.T [N,49]
    att = softmax_s(f)
    out = att @ vis                          [N,512]

Layout: visual rows (n,s) are transposed on the PE into visT[d,(n,s)]
column blocks of 490 (10 n's), so the d/e contractions run as full-width
matmuls in float32r (1 cycle/row on the PE, fp32 storage).  The softmax
runs unnormalized on the [1, 490] f-row; 1/Z is folded into the final
output transpose as a per-partition scale.
"""

import numpy as np

try:
    import concourse.bass as bass
except ImportError:
    import sys as _sys
    for _p in ("/opt/trn_rl_repo", "/root/.axon_site/_ro/trn_rl_repo"):
        if _p not in _sys.path:
            _sys.path.insert(0, _p)
    import concourse.bass as bass
import concourse.mybir as mybir
import concourse.tile as tile
from concourse import bacc

F32 = mybir.dt.float32
F32R = mybir.dt.float32r
AX = mybir.AxisListType
ALU = mybir.AluOpType
AF = mybir.ActivationFunctionType

NCORES = 8
B, T, S, D, E, A = 256, 10, 49, 512, 512, 128
NB = 10              # n's per column block
CB = NB * S          # 490 columns per block


def _r(ap):
    return ap.bitcast(F32R)


def _tr(nc, out, in_, ident):
    nc.tensor.transpose(out.bitcast(ident.dtype), in_, ident)


def build_module(n_n):
    """Build the Bass module for one core handling n_n (b,t) rows."""
    assert n_n % NB == 0
    rows = n_n * S
    nblk = n_n // NB
    n_rt = (rows + 127) // 128           # 128-row visual tiles
    n_nt = (n_n + 127) // 128            # 128-row n tiles (a-path / epilogue)

    nc = bacc.Bacc("TRN2", debug=False)

    aud_d = nc.dram_tensor("audio", [n_n, A], F32R, kind="ExternalInput").ap()
    vis_d = nc.dram_tensor("visual", [rows, D], F32R, kind="ExternalInput").ap()
    wvt_d = nc.dram_tensor("WvT", [128, 4, E], F32R, kind="ExternalInput").ap()
    wat_d = nc.dram_tensor("WaT", [128, E], F32R, kind="ExternalInput").ap()
    aat_d = nc.dram_tensor("AaT", [128, 4, 64], F32R, kind="ExternalInput").ap()
    avt_d = nc.dram_tensor("AvT", [128, 4, S], F32R, kind="ExternalInput").ap()
    aft_d = nc.dram_tensor("AfT", [S, 1], F32R, kind="ExternalInput").ap()
    ba_d = nc.dram_tensor("ba_l", [128, 4], F32, kind="ExternalInput").ap()
    bv_d = nc.dram_tensor("bv_l", [128, 4], F32, kind="ExternalInput").ap()
    idn_d = nc.dram_tensor("ident", [128, 128], F32R, kind="ExternalInput").ap()
    idf_d = nc.dram_tensor("identf", [128, 128], F32, kind="ExternalInput").ap()
    one_d = nc.dram_tensor("ones", [1, 128], F32R, kind="ExternalInput").ap()
    out_d = nc.dram_tensor("out", [n_n, D], F32, kind="ExternalOutput").ap()

    with tile.TileContext(nc) as tc, \
         tc.tile_pool(name="consts", bufs=1) as cp, \
         tc.tile_pool(name="vload", bufs=4) as vp, \
         tc.tile_pool(name="visT", bufs=6) as vtp, \
         tc.tile_pool(name="work", bufs=3) as wp, \
         tc.tile_pool(name="t7p", bufs=2) as t7p, \
         tc.tile_pool(name="dram", bufs=1, space="DRAM") as dp, \
         tc.tile_pool(name="ps_tr", bufs=2, space="PSUM") as ptr, \
         tc.tile_pool(name="ps_mm", bufs=3, space="PSUM") as pmm, \
         tc.tile_pool(name="ps_vs", bufs=2, space="PSUM") as pvs, \
         tc.tile_pool(name="ps_ft", bufs=1, space="PSUM") as pft:

        # ---------- constants ----------
        wvt = cp.tile([128, 4, E], F32R, tag="wvt")
        nc.sync.dma_start(wvt[:], wvt_d)
        wat = cp.tile([128, E], F32R, tag="wat")
        nc.sync.dma_start(wat[:], wat_d)
        aat = cp.tile([128, 4, 64], F32R, tag="aat")
        nc.sync.dma_start(aat[:], aat_d)
        avt = cp.tile([128, 4, S], F32R, tag="avt")
        nc.sync.dma_start(avt[:], avt_d)
        aft = cp.tile([S, 1], F32R, tag="aft")
        nc.sync.dma_start(aft[:], aft_d)
        ba = cp.tile([128, 4], F32, tag="ba")
        nc.sync.dma_start(ba[:], ba_d)
        bv = cp.tile([128, 4], F32, tag="bv")
        nc.sync.dma_start(bv[:], bv_d)
        idn = cp.tile([128, 128], F32R, tag="idn")
        nc.sync.dma_start(idn[:], idn_d)
        idf = cp.tile([128, 128], F32, tag="idf")
        nc.sync.dma_start(idf[:], idf_d)
        ones = cp.tile([1, 128], F32R, tag="ones")
        nc.sync.dma_start(ones[:], one_d)

        audT = cp.tile([128, n_n], F32R, tag="audT")     # audio.T  [a, n]
        atT = cp.tile([128, 4, n_n], F32R, tag="atT")    # a_t.T    [e, n]
        asr = cp.tile([1, rows], F32R, tag="asr")        # a_s row  [(n,s)]
        rinv = cp.tile([1, n_n], F32, tag="rinv")       # 1/Z per n
        outT = cp.tile([128, 4, n_n], F32, tag="outT")  # out.T    [d, n]

        # ---------- a-path prologue ----------
        for it in range(n_nt):
            n0 = it * 128
            nr = min(128, n_n - n0)
            an = wp.tile([128, A], F32R, tag="an")
            nc.sync.dma_start(an[:nr, :], aud_d[n0:n0 + nr, :])
            ps = ptr.tile([128, 128], F32, tag="tr")
            _tr(nc, ps[:, :nr], an[:nr, :], idn[:nr, :nr])
            nc.scalar.copy(audT[:, n0:n0 + nr], ps[:, :nr])

        for eo in range(4):
            ps = pmm.tile([128, max(CB, n_n)], F32, tag="mm")
            nc.tensor.matmul(ps[:, :n_n], wat[:, eo * 128:(eo + 1) * 128],
                             audT[:], start=True, stop=True)
            nc.scalar.activation(atT[:, eo, :], ps[:, :n_n], AF.Relu,
                                 bias=ba[:, eo:eo + 1])

        as_dram = dp.tile([1, rows], F32R, tag="asd")
        for it in range(n_nt):
            n0 = it * 128
            nr = min(128, n_n - n0)
            psa = pvs.tile([128, CB], F32, tag="vs")
            for eo in range(4):
                nc.tensor.matmul(psa[:nr, :64], atT[:, eo, n0:n0 + nr],
                                 aat[:, eo, :],
                                 start=(eo == 0), stop=(eo == 3))
            asn = wp.tile([128, S], F32R, tag="asn")
            nc.scalar.copy(asn[:nr, :], psa[:nr, :S])
            dst = as_dram[0:1, n0 * S:(n0 + nr) * S]
            nc.sync.dma_start(dst.rearrange("one (n s) -> (one n) s", s=S),
                              asn[:nr, :])
        nc.sync.dma_start(asr[:], as_dram[:])

        # ---------- main loop: visual transpose + per-block pipeline ----------
        visT = {}

        def get_visT(b):
            if b not in visT:
                visT[b] = vtp.tile([128, 4, CB], F32R, tag="visT",
                                   name=f"visT{b}")
            return visT[b]

        def do_rtile(t):
            r0 = t * 128
            rt = min(128, rows - r0)
            vn = vp.tile([128, D], F32R, tag="vn")
            nc.sync.dma_start(vn[:rt, :], vis_d[r0:r0 + rt, :])
            b0, b1 = r0 // CB, (r0 + rt - 1) // CB
            ps = ptr.tile([128, 4, 128], F32, tag="tr")
            for do in range(4):
                _tr(nc, ps[:, do, :rt], vn[:rt, do * 128:(do + 1) * 128],
                    idn[:rt, :rt])
            cp_op = nc.scalar.copy if t % 2 == 0 else nc.vector.tensor_copy
            for bb in range(b0, b1 + 1):
                lo = max(r0, bb * CB)
                hi = min(r0 + rt, (bb + 1) * CB)
                cp_op(get_visT(bb)[:, :, lo - bb * CB:hi - bb * CB],
                      ps[:, :, lo - r0:hi - r0])

        t_next = 0
        for b in range(nblk):
            t_end = (b * CB + CB - 1) // 128
            while t_next <= t_end and t_next < n_rt:
                do_rtile(t_next)
                t_next += 1
            vb = get_visT(b)

            # v_t.T = relu(Wv @ vis.T + bv)   [e, col]
            vt = wp.tile([128, 4, CB], F32R, tag="vtT")
            for eo in range(4):
                ps = pmm.tile([128, max(CB, n_n)], F32, tag="mm")
                for do in range(4):
                    nc.tensor.matmul(ps[:, :CB],
                                     wvt[:, do, eo * 128:(eo + 1) * 128],
                                     vb[:, do, :],
                                     start=(do == 0), stop=(do == 3))
                nc.scalar.activation(vt[:, eo, :], ps[:, :CB], AF.Relu,
                                     bias=bv[:, eo:eo + 1])

            # v_s.T + a_s  [f, col]
            psv = pvs.tile([128, CB], F32, tag="vs")
            for eo in range(4):
                nc.tensor.matmul(psv[:S, :], avt[:, eo, :],
                                 vt[:, eo, :], start=(eo == 0), stop=False)
            nc.tensor.matmul(psv[:S, :], ones[0:1, 0:S],
                             asr[0:1, b * CB:(b + 1) * CB],
                             start=False, stop=True)

            th = wp.tile([S, CB], F32R, tag="tanh")
            nc.scalar.activation(th[:], psv[:S, :], AF.Tanh)

            # f row = Af @ tanh  [1, col]
            psf = pft.tile([1, CB], F32, tag="ft")
            nc.tensor.matmul(psf[:], aft[:], th[:], start=True, stop=True)

            # unnormalized softmax: e = exp(f); Z per n; att = e (scaled later)
            ex = wp.tile([1, CB], F32R, tag="exp")
            nc.scalar.activation(ex[:], psf[:], AF.Exp)
            sm = wp.tile([1, NB], F32, tag="ssum")
            nc.vector.reduce_sum(sm[:], ex[:].bitcast(F32).rearrange("p (n s) -> p n s", n=NB),
                                 axis=AX.X)
            nc.vector.reciprocal(rinv[0:1, b * NB:(b + 1) * NB], sm[:])

            # broadcast e across 128 partitions via ones-matmul
            psb = pmm.tile([128, max(CB, n_n)], F32, tag="mm")
            nc.tensor.matmul(psb[:, :CB], ones[0:1, :], ex[:],
                             start=True, stop=True)
            ab = wp.tile([128, CB], F32, tag="attb")
            nc.scalar.copy(ab[:], psb[:, :CB])

            # out.T[d, n] += sum_s visT[d,(n,s)] * e[(n,s)]
            t7 = t7p.tile([128, 4, CB], F32, tag="t7")
            for do in range(4):
                nc.vector.tensor_tensor(t7[:, do, :], vb[:, do, :].bitcast(F32),
                                        ab[:], ALU.mult)
            nc.vector.reduce_sum(
                outT[:, :, b * NB:(b + 1) * NB],
                t7[:].rearrange("p f (n s) -> p f n s", n=NB), axis=AX.X)
            del visT[b]

        # ---------- epilogue: transpose out.T back, scale by 1/Z, store ----------
        for it in range(n_nt):
            n0 = it * 128
            nr = min(128, n_n - n0)
            psr = ptr.tile([128, 128], F32, tag="tr")
            _tr(nc, psr[:nr, 0:1], rinv[0:1, n0:n0 + nr], idf[0:1, 0:1])
            rin = wp.tile([128, 1], F32, tag="rin")
            nc.vector.tensor_copy(rin[:nr, :], psr[:nr, 0:1])
            on = wp.tile([128, D], F32, tag="on")
            for do in range(4):
                pso = ptr.tile([128, 128], F32, tag="tr")
                _tr(nc, pso[:nr, :], outT[:, do, n0:n0 + nr], idf[:, :])
                nc.scalar.activation(on[:nr, do * 128:(do + 1) * 128],
                                     pso[:nr, :], AF.Copy, scale=rin[:nr, 0:1])
            nc.sync.dma_start(out_d[n0:n0 + nr, :], on[:nr, :])

    nc.finalize()
    return nc


def prep_consts(Wa, ba, Wv, bv, Aa, Av, Af):
    f = np.float32
    c = {}
    c["WvT"] = np.ascontiguousarray(
        Wv.T.reshape(4, 128, E).transpose(1, 0, 2)).astype(f)
    c["WaT"] = np.ascontiguousarray(Wa.T).astype(f)
    aat = np.zeros((128, 4, 64), f)
    aat[:, :, :S] = Aa.T.reshape(4, 128, S).transpose(1, 0, 2)
    c["AaT"] = aat
    c["AvT"] = np.ascontiguousarray(
        Av.T.reshape(4, 128, S).transpose(1, 0, 2)).astype(f)
    c["AfT"] = np.ascontiguousarray(Af.reshape(1, S).T).astype(f)
    c["ba_l"] = np.ascontiguousarray(ba.reshape(4, 128).T).astype(f)
    c["bv_l"] = np.ascontiguousarray(bv.reshape(4, 128).T).astype(f)
    c["ident"] = np.eye(128, dtype=f)
    c["identf"] = np.eye(128, dtype=f)
    c["ones"] = np.ones((1, 128), dtype=f)
    return c


_CACHE = {}


def kernel(audio, visual, Wa, ba, Wv, bv, Aa, Av, Af):
    from concourse.bass_utils import run_bass_kernel_spmd

    audio = np.asarray(audio, np.float32)
    visual = np.asarray(visual, np.float32)
    n_n = (B // NCORES) * T  # 320

    if "nc" not in _CACHE:
        _CACHE["nc"] = build_module(n_n)
    nc = _CACHE["nc"]

    consts = prep_consts(np.asarray(Wa, np.float32), np.asarray(ba, np.float32),
                         np.asarray(Wv, np.float32), np.asarray(bv, np.float32),
                         np.asarray(Aa, np.float32), np.asarray(Av, np.float32),
                         np.asarray(Af, np.float32))
    bs = B // NCORES
    in_maps = []
    for c in range(NCORES):
        m = dict(consts)
        m["audio"] = np.ascontiguousarray(
            audio[c * bs:(c + 1) * bs].reshape(n_n, A))
        m["visual"] = np.ascontiguousarray(
            visual[c * bs:(c + 1) * bs].reshape(n_n * S, D))
        in_maps.append(m)

    res = run_bass_kernel_spmd(nc, in_maps, core_ids=list(range(NCORES)))
    _CACHE["last_res"] = res
    out = np.concatenate(
        [r["out"].reshape(bs, T, D) for r in res.results], axis=0)
    return out.astype(np.float32)



# revision 2
# speedup vs baseline: 1.3544x; 1.3544x over previous
"""Trainium2 Bass kernel for nn_AttentionNet (audio-visual attention).

Data-parallel across 8 NeuronCores: batch B=256 split 32 per core, i.e.
n_n = 320 (b,t) rows and 320*49 = 15680 visual rows per core.

Per-core math (n indexes the 320 rows, s in [0,49), d/e in [0,512)):
    a_t = relu(audio @ Wa.T + ba)            [N,512]
    v_t = relu(vis @ Wv.T + bv)              [N,49,512]
    a_s = a_t @ Aa.T                         [N,49]
    v_s = v_t @ Av.T                         [N,49,49]
    f   = (tanh(a_s[:,:,None] + v_s)) @ Af.T [N,49]
    att = softmax_s(f)
    out = att @ vis                          [N,512]

Key layout decision: the visual tensor is pre-transposed ON THE HOST into
a blocked bf16 layout visT[b, p, do, j] = vis[b*490 + j, do*128 + p] so the
device never transposes the 32 MB visual tensor (saves ~40us of PE transposes
and ~75us of PSUM-drain copies per core) and DMA traffic is halved (bf16).
All heavy matmuls run in bf16 (1 cycle/row).  The softmax runs unnormalized;
1/Z is folded into the final output transpose as a per-partition scale.
"""

import numpy as np

try:
    import concourse.bass as bass
except ImportError:
    import sys as _sys
    for _p in ("/opt/trn_rl_repo", "/root/.axon_site/_ro/trn_rl_repo"):
        if _p not in _sys.path:
            _sys.path.insert(0, _p)
    import concourse.bass as bass
import concourse.mybir as mybir
import concourse.tile as tile
from concourse import bacc
from concourse.bass import broadcast_tensor_aps

F32 = mybir.dt.float32
BF16 = mybir.dt.bfloat16
AX = mybir.AxisListType
ALU = mybir.AluOpType
AF = mybir.ActivationFunctionType

NCORES = 8
B, T, S, D, E, A = 256, 10, 49, 512, 512, 128
NB = 10              # n's per column block
CB = NB * S          # 490 columns per block
NBLK = (B // NCORES) * T // NB   # 32 blocks per core


def build_module(n_n, has_ba=False, has_bv=False):
    """Build the Bass module for one core handling n_n (b,t) rows."""
    assert n_n == NB * NBLK
    rows = n_n * S
    n_nt = (n_n + 127) // 128            # 128-row n tiles (a-path / epilogue)

    nc = bacc.Bacc("TRN2", debug=False)

    vist_d = nc.dram_tensor("visT", [NBLK, 128, 4, CB], BF16,
                            kind="ExternalInput").ap()
    aud_d = nc.dram_tensor("audio", [n_n, A], BF16, kind="ExternalInput").ap()
    wvt_d = nc.dram_tensor("WvT", [128, 4, E], BF16, kind="ExternalInput").ap()
    wat_d = nc.dram_tensor("WaT", [128, 4, 128], BF16, kind="ExternalInput").ap()
    aat_d = nc.dram_tensor("AaT", [128, 4, S], BF16, kind="ExternalInput").ap()
    avt_d = nc.dram_tensor("AvT", [128, 4, S], BF16, kind="ExternalInput").ap()
    aft_d = nc.dram_tensor("AfT", [S, 1], BF16, kind="ExternalInput").ap()
    bal_d = nc.dram_tensor("ba_l", [128, 4], F32, kind="ExternalInput").ap()
    bvl_d = nc.dram_tensor("bv_l", [128, 4], F32, kind="ExternalInput").ap()
    idn_d = nc.dram_tensor("ident", [128, 128], BF16, kind="ExternalInput").ap()
    idf_d = nc.dram_tensor("identf", [128, 128], F32, kind="ExternalInput").ap()
    one_d = nc.dram_tensor("ones", [1, 128], BF16, kind="ExternalInput").ap()
    out_d = nc.dram_tensor("out", [n_n, D], F32, kind="ExternalOutput").ap()

    with tile.TileContext(nc) as tc, \
         tc.tile_pool(name="consts", bufs=1) as cp, \
         tc.tile_pool(name="vload", bufs=4) as vp, \
         tc.tile_pool(name="work", bufs=2) as wp, \
         tc.tile_pool(name="dram", bufs=1, space="DRAM") as dp, \
         tc.tile_pool(name="ps_mm", bufs=2, space="PSUM") as pmm, \
         tc.tile_pool(name="ps_vs", bufs=2, space="PSUM") as pvs, \
         tc.tile_pool(name="ps_ft", bufs=1, space="PSUM") as pft, \
         tc.tile_pool(name="ps_sb", bufs=1, space="PSUM") as psb:

        # ---------- constants ----------
        wvt = cp.tile([128, 4, E], BF16, tag="wvt")
        nc.sync.dma_start(wvt[:], wvt_d)
        wat = cp.tile([128, 4, 128], BF16, tag="wat")
        nc.sync.dma_start(wat[:], wat_d)
        aat = cp.tile([128, 4, S], BF16, tag="aat")
        nc.sync.dma_start(aat[:], aat_d)
        avt = cp.tile([128, 4, S], BF16, tag="avt")
        nc.sync.dma_start(avt[:], avt_d)
        aft = cp.tile([S, 1], BF16, tag="aft")
        nc.sync.dma_start(aft[:], aft_d)
        ba = cp.tile([128, 4], F32, tag="ba")
        nc.sync.dma_start(ba[:], bal_d)
        bv = cp.tile([128, 4], F32, tag="bv")
        nc.sync.dma_start(bv[:], bvl_d)
        idn = cp.tile([128, 128], BF16, tag="idn")
        nc.sync.dma_start(idn[:], idn_d)
        idf = cp.tile([128, 128], F32, tag="idf")
        nc.sync.dma_start(idf[:], idf_d)
        ones = cp.tile([1, 128], BF16, tag="ones")
        nc.sync.dma_start(ones[:], one_d)

        audT = cp.tile([128, n_n], BF16, tag="audT")    # audio.T  [a, n]
        atT = cp.tile([128, 4, n_n], BF16, tag="atT")   # a_t.T    [e, n]
        asr = cp.tile([1, rows], BF16, tag="asr")       # a_s row  [(n,s)]
        rinv = cp.tile([1, n_n], F32, tag="rinv")       # 1/Z per n
        outT = cp.tile([128, 4, n_n], F32, tag="outT")  # out.T    [d, n]

        # ---------- a-path prologue ----------
        for it in range(n_nt):
            n0 = it * 128
            nr = min(128, n_n - n0)
            an = wp.tile([128, A], BF16, tag="an")
            nc.sync.dma_start(an[:nr, :], aud_d[n0:n0 + nr, :])
            ps = pvs.tile([128, 128], BF16, tag="vs")
            nc.tensor.transpose(ps[:, :nr], an[:nr, :], idn[:nr, :nr])
            nc.vector.tensor_copy(audT[:, n0:n0 + nr], ps[:, :nr])

        for half in range(2):
            ps = pmm.tile([128, 2, 512], F32, tag="mm")
            for i in range(2):
                eo = half * 2 + i
                nc.tensor.matmul(ps[:, i, :n_n], wat[:, eo, :], audT[:],
                                 start=True, stop=True)
            if has_ba:
                for i in range(2):
                    eo = half * 2 + i
                    nc.scalar.activation(atT[:, eo, :], ps[:, i, :n_n],
                                         AF.Relu, bias=ba[:, eo:eo + 1])
            else:
                nc.scalar.activation(atT[:, half * 2:half * 2 + 2, :],
                                     ps[:, :, :n_n], AF.Relu)

        as_dram = dp.tile([1, rows], BF16, tag="asd")
        for it in range(n_nt):
            n0 = it * 128
            nr = min(128, n_n - n0)
            psa = pvs.tile([128, 64], F32, tag="vs")
            for eo in range(4):
                nc.tensor.matmul(psa[:nr, :S], atT[:, eo, n0:n0 + nr],
                                 aat[:, eo, :],
                                 start=(eo == 0), stop=(eo == 3))
            asn = wp.tile([128, S], BF16, tag="asn")
            nc.vector.tensor_copy(asn[:nr, :], psa[:nr, :S])
            dst = as_dram[0:1, n0 * S:(n0 + nr) * S]
            nc.sync.dma_start(dst.rearrange("one (n s) -> (one n) s", s=S),
                              asn[:nr, :])
        nc.sync.dma_start(asr[:], as_dram[:])

        # ---------- main loop ----------
        for b in range(NBLK):
            vis = vp.tile([128, 4, CB], BF16, tag="vis")
            nc.sync.dma_start(vis[:], vist_d[b])

            # v_t.T = relu(Wv @ vis.T + bv)   [e, col]
            vt = wp.tile([128, 4, CB], BF16, tag="vt")
            for half in range(2):
                ps = pmm.tile([128, 2, 512], F32, tag="mm")
                for i in range(2):
                    eo = half * 2 + i
                    for do in range(4):
                        nc.tensor.matmul(ps[:, i, :CB],
                                         wvt[:, do, eo * 128:(eo + 1) * 128],
                                         vis[:, do, :],
                                         start=(do == 0), stop=(do == 3))
                if has_bv:
                    for i in range(2):
                        eo = half * 2 + i
                        nc.scalar.activation(vt[:, eo, :], ps[:, i, :CB],
                                             AF.Relu, bias=bv[:, eo:eo + 1])
                else:
                    nc.scalar.activation(vt[:, half * 2:half * 2 + 2, :],
                                         ps[:, :, :CB], AF.Relu)

            # v_s.T + a_s  [f, col]
            psv = pvs.tile([64, 512], F32, tag="vs")
            for eo in range(4):
                nc.tensor.matmul(psv[:S, :CB], avt[:, eo, :],
                                 vt[:, eo, :], start=(eo == 0), stop=False)
            nc.tensor.matmul(psv[:S, :CB], ones[0:1, 0:S],
                             asr[0:1, b * CB:(b + 1) * CB],
                             start=False, stop=True)

            th = wp.tile([S, CB], BF16, tag="th")
            nc.scalar.activation(th[:], psv[:S, :CB], AF.Tanh)

            # f row = Af @ tanh  [1, col]
            psf = pft.tile([1, 512], F32, tag="ft")
            nc.tensor.matmul(psf[0:1, :CB], aft[:], th[:],
                             start=True, stop=True)

            # unnormalized softmax: e = exp(f); Z per n (normalize at end)
            ex = wp.tile([1, CB], BF16, tag="ex")
            nc.scalar.activation(ex[:], psf[0:1, :CB], AF.Exp)
            zs = wp.tile([1, NB], F32, tag="zs")
            nc.vector.reduce_sum(zs[:], ex[:].rearrange("p (n s) -> p n s", s=S),
                                 axis=AX.X)
            nc.vector.reciprocal(rinv[0:1, b * NB:(b + 1) * NB], zs[:])

            # broadcast e across 128 partitions via ones-matmul
            pb = psb.tile([128, 512], F32, tag="sb")
            nc.tensor.matmul(pb[:, :CB], ones[0:1, :], ex[:],
                             start=True, stop=True)
            ab = wp.tile([128, CB], BF16, tag="ab")
            nc.scalar.copy(ab[:], pb[:, :CB])

            # out.T[d, n] += sum_s visT[d,(n,s)] * e[(n,s)]
            t7 = wp.tile([128, 4, CB], BF16, tag="t7")
            in0, in1 = broadcast_tensor_aps(vis[:], ab[:].rearrange(
                "p (one c) -> p one c", one=1))
            nc.vector.tensor_tensor(t7[:], in0, in1, ALU.mult)
            nc.vector.reduce_sum(
                outT[:, :, b * NB:(b + 1) * NB],
                t7[:].rearrange("p f (n s) -> p f n s", s=S), axis=AX.X)

        # ---------- epilogue: transpose out.T back, scale by 1/Z, store ----------
        for it in range(n_nt):
            n0 = it * 128
            nr = min(128, n_n - n0)
            psr = pvs.tile([128, 128], F32, tag="vs")
            nc.tensor.transpose(psr[:nr, 0:1], rinv[0:1, n0:n0 + nr],
                                idf[0:1, 0:1])
            rin = wp.tile([128, 1], F32, tag="rin")
            nc.vector.tensor_copy(rin[:nr, :], psr[:nr, 0:1])
            on = wp.tile([128, D], F32, tag="on")
            for do in range(4):
                pso = pmm.tile([128, 2, 512], F32, tag="mm")
                nc.tensor.transpose(pso[:nr, 0, :128], outT[:, do, n0:n0 + nr],
                                    idf[:, :])
                nc.scalar.activation(on[:nr, do * 128:(do + 1) * 128],
                                     pso[:nr, 0, :128], AF.Copy,
                                     scale=rin[:nr, 0:1])
            nc.sync.dma_start(out_d[n0:n0 + nr, :], on[:nr, :])

    nc.finalize()
    return nc


def prep_consts(Wa, ba, Wv, bv, Aa, Av, Af):
    import ml_dtypes
    bf = ml_dtypes.bfloat16
    f = np.float32
    c = {}
    # wvt[p, do, e] = Wv[e, do*128+p]
    c["WvT"] = np.ascontiguousarray(
        Wv.T.reshape(4, 128, E).transpose(1, 0, 2)).astype(bf)
    # wat[a, eo, m] = Wa[eo*128+m, a]
    c["WaT"] = np.ascontiguousarray(
        Wa.T.reshape(A, 4, 128)).astype(bf)
    # aat[p, eo, s] = Aa[s, eo*128+p]
    c["AaT"] = np.ascontiguousarray(
        Aa.T.reshape(4, 128, S).transpose(1, 0, 2)).astype(bf)
    c["AvT"] = np.ascontiguousarray(
        Av.T.reshape(4, 128, S).transpose(1, 0, 2)).astype(bf)
    c["AfT"] = np.ascontiguousarray(Af.reshape(1, S).T).astype(bf)
    c["ba_l"] = np.ascontiguousarray(ba.reshape(4, 128).T).astype(f)
    c["bv_l"] = np.ascontiguousarray(bv.reshape(4, 128).T).astype(f)
    c["ident"] = np.eye(128, dtype=np.float32).astype(bf)
    c["identf"] = np.eye(128, dtype=f)
    c["ones"] = np.ones((1, 128), dtype=np.float32).astype(bf)
    return c


_CACHE = {}


def kernel(audio, visual, Wa, ba, Wv, bv, Aa, Av, Af):
    from concourse.bass_utils import run_bass_kernel_spmd
    import ml_dtypes
    bf = ml_dtypes.bfloat16

    audio = np.asarray(audio, np.float32)
    visual = np.asarray(visual, np.float32)
    ba = np.asarray(ba, np.float32)
    bv = np.asarray(bv, np.float32)
    n_n = (B // NCORES) * T  # 320

    has_ba = bool(np.any(ba))
    has_bv = bool(np.any(bv))
    key = ("nc", has_ba, has_bv)
    if key not in _CACHE:
        _CACHE[key] = build_module(n_n, has_ba, has_bv)
    nc = _CACHE[key]

    consts = prep_consts(np.asarray(Wa, np.float32), ba,
                         np.asarray(Wv, np.float32), bv,
                         np.asarray(Aa, np.float32),
                         np.asarray(Av, np.float32),
                         np.asarray(Af, np.float32))
    bs = B // NCORES
    vis_bf = visual.astype(bf)          # one cast for the full tensor
    aud_bf = audio.astype(bf)
    in_maps = []
    for c in range(NCORES):
        m = dict(consts)
        m["audio"] = np.ascontiguousarray(
            aud_bf[c * bs:(c + 1) * bs].reshape(n_n, A))
        # visT[b, p, do, j] = vis[b*CB + j, do*128 + p]
        v = vis_bf[c * bs:(c + 1) * bs].reshape(NBLK, CB, 4, 128)
        m["visT"] = np.ascontiguousarray(v.transpose(0, 3, 2, 1))
        in_maps.append(m)

    res = run_bass_kernel_spmd(nc, in_maps, core_ids=list(range(NCORES)))
    _CACHE["last_res"] = res
    out = np.concatenate(
        [r["out"].reshape(bs, T, D) for r in res.results], axis=0)
    return out.astype(np.float32)


# revision 8
# speedup vs baseline: 1.6367x; 1.2084x over previous
"""Trainium2 Bass kernel for nn_AttentionNet (audio-visual attention).

Data-parallel across 8 NeuronCores: batch B=256 split 32 per core, i.e.
n_n = 320 (b,t) rows and 320*49 = 15680 visual rows per core.

Per-core math (n indexes the 320 rows, s in [0,49), d/e in [0,512)):
    a_t = relu(audio @ Wa.T + ba)            [N,512]
    v_t = relu(vis @ Wv.T + bv)              [N,49,512]
    a_s = a_t @ Aa.T                         [N,49]
    v_s = v_t @ Av.T                         [N,49,49]
    f   = (tanh(a_s[:,:,None] + v_s)) @ Af.T [N,49]
    att = softmax_s(f)
    out = att @ vis                          [N,512]

Key layout decision: the visual tensor is pre-transposed ON THE HOST into
a blocked bf16 layout visT[b, p, do, j] = vis[b*490 + j, do*128 + p] so the
device never transposes the 32 MB visual tensor (saves ~40us of PE transposes
and ~75us of PSUM-drain copies per core) and DMA traffic is halved (bf16).
All heavy matmuls run in bf16 (1 cycle/row).  The softmax runs unnormalized;
1/Z is folded into the final output transpose as a per-partition scale.
"""

import numpy as np

try:
    import concourse.bass as bass
except ImportError:
    import sys as _sys
    for _p in ("/opt/trn_rl_repo", "/root/.axon_site/_ro/trn_rl_repo"):
        if _p not in _sys.path:
            _sys.path.insert(0, _p)
    import concourse.bass as bass
import concourse.mybir as mybir
import concourse.tile as tile
from concourse import bacc
from concourse.bass import broadcast_tensor_aps

F32 = mybir.dt.float32
BF16 = mybir.dt.bfloat16
AX = mybir.AxisListType
ALU = mybir.AluOpType
AF = mybir.ActivationFunctionType

NCORES = 8
B, T, S, D, E, A = 256, 10, 49, 512, 512, 128
NB = 10              # n's per column block
CB = NB * S          # 490 columns per block
NBLK = (B // NCORES) * T // NB   # 32 blocks per core

FP8_WV = True        # run the Wv matmul in fp8e4 with DoubleRow
WSCALE = 32.0        # host-side weight scale for fp8 dynamic range


def build_module(n_n, has_ba=False, has_bv=False):
    """Build the Bass module for one core handling n_n (b,t) rows."""
    assert n_n == NB * NBLK
    rows = n_n * S
    n_nt = (n_n + 127) // 128            # 128-row n tiles (a-path / epilogue)

    nc = bacc.Bacc("TRN2", debug=False)

    FP8 = mybir.dt.float8e4
    vist_d = nc.dram_tensor("visT", [NBLK, 128, 4, CB], BF16,
                            kind="ExternalInput").ap()
    if FP8_WV:
        vis8_d = nc.dram_tensor("visT8", [NBLK, 128, 4, CB], FP8,
                                kind="ExternalInput").ap()
        wv8_d = nc.dram_tensor("Wv8", [128, 4, E], FP8,
                               kind="ExternalInput").ap()
    aud_d = nc.dram_tensor("audio", [n_n, A], BF16, kind="ExternalInput").ap()
    wvt_d = nc.dram_tensor("WvT", [128, 4, E], BF16, kind="ExternalInput").ap()
    wat_d = nc.dram_tensor("WaT", [128, 4, 128], BF16, kind="ExternalInput").ap()
    aat_d = nc.dram_tensor("AaT", [128, 4, S], BF16, kind="ExternalInput").ap()
    avt_d = nc.dram_tensor("AvT", [128, 4, S], BF16, kind="ExternalInput").ap()
    aft_d = nc.dram_tensor("AfT", [S, 1], BF16, kind="ExternalInput").ap()
    bal_d = nc.dram_tensor("ba_l", [128, 4], F32, kind="ExternalInput").ap()
    bvl_d = nc.dram_tensor("bv_l", [128, 4], F32, kind="ExternalInput").ap()
    idn_d = nc.dram_tensor("ident", [128, 128], BF16, kind="ExternalInput").ap()
    idf_d = nc.dram_tensor("identf", [128, 128], F32, kind="ExternalInput").ap()
    one_d = nc.dram_tensor("ones", [1, 128], BF16, kind="ExternalInput").ap()
    out_d = nc.dram_tensor("out", [n_n, D], F32, kind="ExternalOutput").ap()

    with tile.TileContext(nc) as tc, \
         tc.tile_pool(name="consts", bufs=1) as cp, \
         tc.tile_pool(name="vload", bufs=4) as vp, \
         tc.tile_pool(name="work", bufs=2) as wp, \
         tc.tile_pool(name="dram", bufs=1, space="DRAM") as dp, \
         tc.tile_pool(name="ps_mm", bufs=2, space="PSUM") as pmm, \
         tc.tile_pool(name="ps_vs", bufs=2, space="PSUM") as pvs, \
         tc.tile_pool(name="ps_ft", bufs=1, space="PSUM") as pft, \
         tc.tile_pool(name="ps_sb", bufs=1, space="PSUM") as psb:

        # ---------- constants ----------
        if FP8_WV:
            wv8 = cp.tile([128, 4, E], FP8, tag="wv8")
            nc.sync.dma_start(wv8[:], wv8_d)
        else:
            wvt = cp.tile([128, 4, E], BF16, tag="wvt")
            nc.sync.dma_start(wvt[:], wvt_d)
        wat = cp.tile([128, 4, 128], BF16, tag="wat")
        nc.sync.dma_start(wat[:], wat_d)
        aat = cp.tile([128, 4, S], BF16, tag="aat")
        nc.sync.dma_start(aat[:], aat_d)
        avt = cp.tile([128, 4, S], BF16, tag="avt")
        nc.sync.dma_start(avt[:], avt_d)
        aft = cp.tile([S, 1], BF16, tag="aft")
        nc.sync.dma_start(aft[:], aft_d)
        ba = cp.tile([128, 4], F32, tag="ba")
        nc.sync.dma_start(ba[:], bal_d)
        bv = cp.tile([128, 4], F32, tag="bv")
        nc.sync.dma_start(bv[:], bvl_d)
        idn = cp.tile([128, 128], BF16, tag="idn")
        nc.sync.dma_start(idn[:], idn_d)
        idf = cp.tile([128, 128], F32, tag="idf")
        nc.sync.dma_start(idf[:], idf_d)
        ones = cp.tile([1, 128], BF16, tag="ones")
        nc.sync.dma_start(ones[:], one_d)

        audT = cp.tile([128, n_n], BF16, tag="audT")    # audio.T  [a, n]
        atT = cp.tile([128, 4, n_n], BF16, tag="atT")   # a_t.T    [e, n]
        asr = cp.tile([1, rows], BF16, tag="asr")       # a_s row  [(n,s)]
        rinv = cp.tile([1, n_n], F32, tag="rinv")       # 1/Z per n
        outT = cp.tile([128, 4, n_n], F32, tag="outT")  # out.T    [d, n]

        # ---------- a-path prologue ----------
        for it in range(n_nt):
            n0 = it * 128
            nr = min(128, n_n - n0)
            an = wp.tile([128, A], BF16, tag="an")
            nc.sync.dma_start(an[:nr, :], aud_d[n0:n0 + nr, :])
            ps = pvs.tile([128, 128], BF16, tag="vs")
            nc.tensor.transpose(ps[:, :nr], an[:nr, :], idn[:nr, :nr])
            nc.vector.tensor_copy(audT[:, n0:n0 + nr], ps[:, :nr])

        for half in range(2):
            ps = pmm.tile([128, 2, 512], F32, tag="mm")
            for i in range(2):
                eo = half * 2 + i
                nc.tensor.matmul(ps[:, i, :n_n], wat[:, eo, :], audT[:],
                                 start=True, stop=True)
            if has_ba:
                for i in range(2):
                    eo = half * 2 + i
                    nc.scalar.activation(atT[:, eo, :], ps[:, i, :n_n],
                                         AF.Relu, bias=ba[:, eo:eo + 1])
            else:
                nc.scalar.activation(atT[:, half * 2:half * 2 + 2, :],
                                     ps[:, :, :n_n], AF.Relu)

        as_dram = dp.tile([1, rows], BF16, tag="asd")
        for it in range(n_nt):
            n0 = it * 128
            nr = min(128, n_n - n0)
            psa = pvs.tile([128, 64], F32, tag="vs")
            for eo in range(4):
                nc.tensor.matmul(psa[:nr, :S], atT[:, eo, n0:n0 + nr],
                                 aat[:, eo, :],
                                 start=(eo == 0), stop=(eo == 3))
            asn = wp.tile([128, S], BF16, tag="asn")
            nc.vector.tensor_copy(asn[:nr, :], psa[:nr, :S])
            dst = as_dram[0:1, n0 * S:(n0 + nr) * S]
            nc.sync.dma_start(dst.rearrange("one (n s) -> (one n) s", s=S),
                              asn[:nr, :])
        nc.sync.dma_start(asr[:], as_dram[:])

        # ---------- main loop ----------
        rscale = (1.0 / WSCALE) if FP8_WV else 1.0
        for b in range(NBLK):
            vis = vp.tile([128, 4, CB], BF16, tag="vis")
            nc.sync.dma_start(vis[:], vist_d[b])
            if FP8_WV:
                vis8 = vp.tile([128, 4, CB], FP8, tag="vis8")
                nc.sync.dma_start(vis8[:], vis8_d[b])

            # v_t.T = relu(Wv @ vis.T + bv)   [e, col]
            vt = wp.tile([128, 4, CB], BF16, tag="vt")
            for half in range(2):
                ps = pmm.tile([128, 2, 512], F32, tag="mm")
                for i in range(2):
                    eo = half * 2 + i
                    if FP8_WV:
                        for dp in range(2):
                            nc.tensor.matmul(
                                ps[:, i, :CB],
                                wv8[:, 2 * dp:2 * dp + 2,
                                    eo * 128:(eo + 1) * 128],
                                vis8[:, 2 * dp:2 * dp + 2, :],
                                start=(dp == 0), stop=(dp == 1),
                                perf_mode=mybir.MatmulPerfMode.DoubleRow)
                    else:
                        for do in range(4):
                            nc.tensor.matmul(
                                ps[:, i, :CB],
                                wvt[:, do, eo * 128:(eo + 1) * 128],
                                vis[:, do, :],
                                start=(do == 0), stop=(do == 3))
                if has_bv:
                    for i in range(2):
                        eo = half * 2 + i
                        nc.scalar.activation(vt[:, eo, :], ps[:, i, :CB],
                                             AF.Relu, bias=bv[:, eo:eo + 1],
                                             scale=rscale)
                else:
                    nc.scalar.activation(vt[:, half * 2:half * 2 + 2, :],
                                         ps[:, :, :CB], AF.Relu, scale=rscale)

            # v_s.T + a_s  [f, col]
            psv = pvs.tile([64, 512], F32, tag="vs")
            for eo in range(4):
                nc.tensor.matmul(psv[:S, :CB], avt[:, eo, :],
                                 vt[:, eo, :], start=(eo == 0), stop=False)
            nc.tensor.matmul(psv[:S, :CB], ones[0:1, 0:S],
                             asr[0:1, b * CB:(b + 1) * CB],
                             start=False, stop=True)

            th = wp.tile([S, CB], BF16, tag="th")
            nc.scalar.activation(th[:], psv[:S, :CB], AF.Tanh)

            # f row = Af @ tanh  [1, col]
            psf = pft.tile([1, 512], F32, tag="ft")
            nc.tensor.matmul(psf[0:1, :CB], aft[:], th[:],
                             start=True, stop=True)

            # unnormalized softmax: e = exp(f); Z per n (normalize at end)
            ex = wp.tile([1, CB], BF16, tag="ex")
            nc.scalar.activation(ex[:], psf[0:1, :CB], AF.Exp)
            zs = wp.tile([1, NB], F32, tag="zs")
            nc.vector.reduce_sum(zs[:], ex[:].rearrange("p (n s) -> p n s", s=S),
                                 axis=AX.X)
            nc.vector.reciprocal(rinv[0:1, b * NB:(b + 1) * NB], zs[:])

            # broadcast e across 128 partitions via ones-matmul
            pb = psb.tile([128, 512], F32, tag="sb")
            nc.tensor.matmul(pb[:, :CB], ones[0:1, :], ex[:],
                             start=True, stop=True)
            ab = wp.tile([128, CB], BF16, tag="ab")
            nc.scalar.copy(ab[:], pb[:, :CB])

            # out.T[d, n] += sum_s visT[d,(n,s)] * e[(n,s)]
            t7 = wp.tile([128, 4, CB], BF16, tag="t7")
            in0, in1 = broadcast_tensor_aps(vis[:], ab[:].rearrange(
                "p (one c) -> p one c", one=1))
            nc.vector.tensor_tensor(t7[:], in0, in1, ALU.mult)
            nc.vector.reduce_sum(
                outT[:, :, b * NB:(b + 1) * NB],
                t7[:].rearrange("p f (n s) -> p f n s", s=S), axis=AX.X)

        # ---------- epilogue: transpose out.T back, scale by 1/Z, store ----------
        for it in range(n_nt):
            n0 = it * 128
            nr = min(128, n_n - n0)
            psr = pvs.tile([128, 128], F32, tag="vs")
            nc.tensor.transpose(psr[:nr, 0:1], rinv[0:1, n0:n0 + nr],
                                idf[0:1, 0:1])
            rin = wp.tile([128, 1], F32, tag="rin")
            nc.vector.tensor_copy(rin[:nr, :], psr[:nr, 0:1])
            on = wp.tile([128, D], F32, tag="on")
            for do in range(4):
                pso = pmm.tile([128, 2, 512], F32, tag="mm")
                nc.tensor.transpose(pso[:nr, 0, :128], outT[:, do, n0:n0 + nr],
                                    idf[:, :])
                nc.scalar.activation(on[:nr, do * 128:(do + 1) * 128],
                                     pso[:nr, 0, :128], AF.Copy,
                                     scale=rin[:nr, 0:1])
            nc.sync.dma_start(out_d[n0:n0 + nr, :], on[:nr, :])

    nc.finalize()
    return nc


def prep_consts(Wa, ba, Wv, bv, Aa, Av, Af):
    import ml_dtypes
    bf = ml_dtypes.bfloat16
    f = np.float32
    c = {}
    # wvt[p, do, e] = Wv[e, do*128+p]
    wvt_host = np.ascontiguousarray(Wv.T.reshape(4, 128, E).transpose(1, 0, 2))
    c["WvT"] = wvt_host.astype(bf)
    if FP8_WV:
        c["Wv8"] = (wvt_host * WSCALE).astype(ml_dtypes.float8_e4m3)
    # wat[a, eo, m] = Wa[eo*128+m, a]
    c["WaT"] = np.ascontiguousarray(
        Wa.T.reshape(A, 4, 128)).astype(bf)
    # aat[p, eo, s] = Aa[s, eo*128+p]
    c["AaT"] = np.ascontiguousarray(
        Aa.T.reshape(4, 128, S).transpose(1, 0, 2)).astype(bf)
    c["AvT"] = np.ascontiguousarray(
        Av.T.reshape(4, 128, S).transpose(1, 0, 2)).astype(bf)
    c["AfT"] = np.ascontiguousarray(Af.reshape(1, S).T).astype(bf)
    c["ba_l"] = np.ascontiguousarray(ba.reshape(4, 128).T).astype(f)
    c["bv_l"] = np.ascontiguousarray(bv.reshape(4, 128).T).astype(f)
    c["ident"] = np.eye(128, dtype=np.float32).astype(bf)
    c["identf"] = np.eye(128, dtype=f)
    c["ones"] = np.ones((1, 128), dtype=np.float32).astype(bf)
    return c


_CACHE = {}


def kernel(audio, visual, Wa, ba, Wv, bv, Aa, Av, Af):
    from concourse.bass_utils import run_bass_kernel_spmd
    import ml_dtypes
    bf = ml_dtypes.bfloat16

    audio = np.asarray(audio, np.float32)
    visual = np.asarray(visual, np.float32)
    ba = np.asarray(ba, np.float32)
    bv = np.asarray(bv, np.float32)
    n_n = (B // NCORES) * T  # 320

    has_ba = bool(np.any(ba))
    has_bv = bool(np.any(bv))
    key = ("nc", has_ba, has_bv)
    if key not in _CACHE:
        _CACHE[key] = build_module(n_n, has_ba, has_bv)
    nc = _CACHE[key]

    consts = prep_consts(np.asarray(Wa, np.float32), ba,
                         np.asarray(Wv, np.float32), bv,
                         np.asarray(Aa, np.float32),
                         np.asarray(Av, np.float32),
                         np.asarray(Af, np.float32))
    bs = B // NCORES
    vis_bf = visual.astype(bf)          # one cast for the full tensor
    aud_bf = audio.astype(bf)
    in_maps = []
    for c in range(NCORES):
        m = dict(consts)
        m["audio"] = np.ascontiguousarray(
            aud_bf[c * bs:(c + 1) * bs].reshape(n_n, A))
        # visT[b, p, do, j] = vis[b*CB + j, do*128 + p]
        v = vis_bf[c * bs:(c + 1) * bs].reshape(NBLK, CB, 4, 128)
        vT = np.ascontiguousarray(v.transpose(0, 3, 2, 1))
        m["visT"] = vT
        if FP8_WV:
            m["visT8"] = vT.astype(ml_dtypes.float8_e4m3)
        in_maps.append(m)

    res = run_bass_kernel_spmd(nc, in_maps, core_ids=list(range(NCORES)))
    _CACHE["last_res"] = res
    out = np.concatenate(
        [r["out"].reshape(bs, T, D) for r in res.results], axis=0)
    return out.astype(np.float32)
